# revision 8
# baseline (speedup 1.0000x reference)
"""Distributed attention layer kernel for 8 TRN2 NeuronCores.

Math (per reference): out = softmax_causal((x@Wq)(x@Wk)^T / 8) @ (x@Wv) @ Wo
with B=4, S=2048, D=1024, H=16 heads of dim 64.

Sharding: head tensor-parallel. Core c owns head pair (2c, 2c+1):
  - Wq/Wk/Wv column-sharded [1024, 128]; statesT replicated [1024, 8192].
  - Each core computes qT/kT/vT for its 2 heads, causal attention in
    S^T layout (kj on partitions, qi on free), softmax denominator via a
    ones-column appended to V (PV matmul row 64 = sum of probs).
  - ctx^T lands in an AllToAll buffer laid out so chunk j = [128 head
    cols, 1024 rows of output-core j]; after the A2A each core holds
    lhsT-ready blocks of ALL 1024 ctx columns for its 1024 rows.
  - Output projection: out_rows = sum_c ctxT_block_c.T @ Wo[128c:...]
    accumulated in PSUM; no AllReduce needed.

Matmul operands are bf16 (PE full rate); accumulation is fp32 in PSUM.
"""

import ml_dtypes
import numpy as np

import concourse.bass as bass
import concourse.mybir as mybir
import concourse.tile as tile
from concourse import bacc
from concourse.masks import make_identity

F32 = mybir.dt.float32
BF16 = mybir.dt.bfloat16
NEG_BIAS = -1e35

B, S, D, H = 4, 2048, 1024, 16
HD = 64
N_CORES = 8


def build_bias(KJ=128, NB=4, QI=512):
    """Causal bias tiles for diagonal blocks, in S^T layout.

    bias[p, i*QI + f] = 0 if (KJ*i + p) <= f else NEG_BIAS
    (key index KJ*i + p attends to query index f within the qi tile).
    """
    bias = np.full((KJ, NB * QI), NEG_BIAS, dtype=np.float32)
    for i in range(NB):
        k = KJ * i + np.arange(KJ)[:, None]
        f = np.arange(QI)[None, :]
        bias[:, i * QI:(i + 1) * QI] = np.where(k <= f, 0.0, NEG_BIAS)
    return bias


def build(b_=B, s_=S, d_=D, n_cores=N_CORES):
    HPC = d_ // n_cores          # head cols per core (2 heads x 64)
    NH = HPC // HD               # heads per core
    R = b_ * s_                  # global rows
    Rc = R // n_cores            # output rows per core
    QI, KJ = 512, 128
    DT = d_ // 128               # D tiles (contraction)
    NB = QI // KJ
    SKJ = s_ // KJ               # kj blocks per (b, h)
    SQI = s_ // QI               # qi tiles per (b, h)
    assert s_ % QI == 0 and Rc % QI == 0 and d_ % 128 == 0

    nc = bacc.Bacc(None, target_bir_lowering=False, debug=False)
    statesT = nc.declare_dram_parameter("statesT", [d_, R], BF16, isOutput=False)
    wq = nc.declare_dram_parameter("wq", [d_, HPC], BF16, isOutput=False)
    wk = nc.declare_dram_parameter("wk", [d_, HPC], BF16, isOutput=False)
    wv = nc.declare_dram_parameter("wv", [d_, HPC], BF16, isOutput=False)
    wo = nc.declare_dram_parameter("wo", [d_, d_], BF16, isOutput=False)
    bias_in = nc.declare_dram_parameter("bias", [KJ, NB * QI], F32, isOutput=False)
    out_ext = nc.declare_dram_parameter("out", [Rc, d_], F32, isOutput=True)

    with tile.TileContext(nc) as tc:
        with tc.tile_pool(name="persist", bufs=1) as pp, \
             tc.tile_pool(name="dram", bufs=1, space="DRAM") as dram:
            a2a_in = dram.tile([n_cores * HPC, Rc], BF16, tag="a2a_in")
            a2a_out = dram.tile([n_cores * HPC, Rc], BF16, tag="a2a_out")

            qT = pp.tile([HPC, R], BF16, tag="qT")
            kT = pp.tile([HPC, R], BF16, tag="kT")
            vp = pp.tile([KJ, b_ * NH * SKJ, HD + 1], BF16, tag="vp")
            w_sb = pp.tile([128, 3, DT, HPC], BF16, tag="w_sb")
            bias_sb = pp.tile([KJ, NB * QI], F32, tag="bias_sb")
            ident = pp.tile([128, 128], BF16, tag="ident")
            ones_sb = pp.tile([1, 128], F32, tag="ones")

            make_identity(nc, ident[:])
            nc.vector.memset(ones_sb[:], 1.0)
            nc.sync.dma_start(out=bias_sb[:], in_=bias_in[:, :])
            for i, w in enumerate([wq, wk, wv]):
                nc.sync.dma_start(
                    out=w_sb[:, i], in_=w[:, :].rearrange("(t p) c -> p t c", p=128))

            # ---- Phase 1: QKV projections -> qT, kT, vT (all [HPC, R]) ----
            with tc.tile_pool(name="vT_pool", bufs=1) as vtp:
                vT = vtp.tile([HPC, R], BF16, tag="vT")
                with tc.tile_pool(name="st_in", bufs=3) as stp, \
                     tc.tile_pool(name="qkv_ps", bufs=3, space="PSUM") as qps:
                    dests = [qT, kT, vT]
                    for ci in range(R // QI):
                        st = stp.tile([128, DT, QI], BF16, tag="st")
                        nc.sync.dma_start(
                            out=st[:],
                            in_=statesT[:, ci * QI:(ci + 1) * QI]
                            .rearrange("(t p) f -> p t f", p=128))
                        for pi in range(3):
                            ps = qps.tile([128, QI], F32, tag="ps")
                            for dd in range(DT):
                                nc.tensor.matmul(
                                    ps[:], w_sb[:, pi, dd], st[:, dd],
                                    start=(dd == 0), stop=(dd == DT - 1))
                            nc.vector.tensor_copy(
                                dests[pi][:, ci * QI:(ci + 1) * QI], ps[:])

                # ---- Phase 2: V' blocks [KJ, HD+1] via PE transpose ----
                with tc.tile_pool(name="tp_ps", bufs=4, space="PSUM") as tpp:
                    nc.vector.memset(vp[:, :, HD], 1.0)
                    for bb in range(b_):
                        for h in range(NH):
                            for kj in range(SKJ):
                                blk = (bb * NH + h) * SKJ + kj
                                ps = tpp.tile([KJ, HD], BF16, tag="tp")
                                nc.tensor.transpose(
                                    ps[:],
                                    vT[h * HD:(h + 1) * HD,
                                       bb * s_ + kj * KJ: bb * s_ + (kj + 1) * KJ],
                                    ident[h * HD:(h + 1) * HD,
                                          h * HD:(h + 1) * HD])
                                nc.vector.tensor_copy(vp[:, blk, 0:HD], ps[:])

            # ---- Phase 3: causal attention per (b, h, qi) ----
            with tc.tile_pool(name="s_ps", bufs=3, space="PSUM") as sps, \
                 tc.tile_pool(name="ctx_ps", bufs=2, space="PSUM") as cps, \
                 tc.tile_pool(name="b_ps", bufs=2, space="PSUM") as bps, \
                 tc.tile_pool(name="pt_sb", bufs=4) as ptp, \
                 tc.tile_pool(name="misc_sb", bufs=3) as msp:
                for bb in range(b_):
                    for h in range(NH):
                        for qi in range(SQI):
                            ctx = cps.tile([HD + 1, QI], F32, tag="ctx")
                            nkj = NB * (qi + 1)
                            for kj in range(nkj):
                                ps = sps.tile([KJ, QI], F32, tag="s")
                                nc.tensor.matmul(
                                    ps[:],
                                    kT[h * HD:(h + 1) * HD,
                                       bb * s_ + kj * KJ: bb * s_ + (kj + 1) * KJ],
                                    qT[h * HD:(h + 1) * HD,
                                       bb * s_ + qi * QI: bb * s_ + (qi + 1) * QI],
                                    start=True, stop=True)
                                di = kj - NB * qi
                                if di >= 0:
                                    nc.vector.tensor_add(
                                        ps[:], ps[:],
                                        bias_sb[:, di * QI:(di + 1) * QI])
                                pt = ptp.tile([KJ, QI], BF16, tag="pt")
                                nc.scalar.activation(
                                    pt[:], ps[:],
                                    mybir.ActivationFunctionType.Exp,
                                    scale=float(1.0 / np.sqrt(HD)))
                                blk = (bb * NH + h) * SKJ + kj
                                nc.tensor.matmul(
                                    ctx[:], vp[:, blk], pt[:],
                                    start=(kj == 0), stop=(kj == nkj - 1))
                            # softmax denominator: ctx row HD holds sum(P)
                            recip = msp.tile([1, QI], F32, tag="recip")
                            nc.vector.reciprocal(recip[:], ctx[HD:HD + 1, :])
                            bc = bps.tile([128, QI], F32, tag="bc")
                            nc.tensor.matmul(bc[:], ones_sb[:], recip[:],
                                             start=True, stop=True)
                            bcs = msp.tile([128, QI], F32, tag="bcs")
                            nc.scalar.copy(bcs[:], bc[:])
                            ctxT = msp.tile([HD, QI], BF16, tag="ctxT")
                            nc.vector.tensor_mul(ctxT[:], ctx[0:HD, :], bcs[0:HD, :])
                            g0 = bb * s_ + qi * QI
                            j, r0 = g0 // Rc, g0 % Rc
                            nc.sync.dma_start(
                                out=a2a_in[j * HPC + h * HD: j * HPC + (h + 1) * HD,
                                           r0:r0 + QI],
                                in_=ctxT[:])

            # ---- Phase 4: AllToAll (head shard -> row shard) + out proj ----
            nc.gpsimd.collective_compute(
                "AllToAll", mybir.AluOpType.bypass,
                replica_groups=[list(range(n_cores))],
                ins=[a2a_in[:].opt()], outs=[a2a_out[:].opt()])
            with tc.tile_pool(name="slab", bufs=1) as slp, \
                 tc.tile_pool(name="o_ps", bufs=3, space="PSUM") as ops, \
                 tc.tile_pool(name="o_sb", bufs=3) as osp:
                wo_sb = slp.tile([128, DT, d_], BF16, tag="wo_sb")
                nc.sync.dma_start(
                    out=wo_sb[:], in_=wo[:, :].rearrange("(t p) n -> p t n", p=128))
                slab = slp.tile([HPC, n_cores, Rc], BF16, tag="slab")
                nc.sync.dma_start(
                    out=slab[:],
                    in_=a2a_out[:].rearrange("(c p) f -> p c f", p=HPC))
                for m in range(Rc // 128):
                    for n in range(d_ // QI):
                        ps = ops.tile([128, QI], F32, tag="o")
                        for c in range(n_cores):
                            nc.tensor.matmul(
                                ps[:],
                                slab[:, c, m * 128:(m + 1) * 128],
                                wo_sb[:, c, n * QI:(n + 1) * QI],
                                start=(c == 0), stop=(c == n_cores - 1))
                        ob = osp.tile([128, QI], F32, tag="ob")
                        nc.vector.tensor_copy(ob[:], ps[:])
                        nc.sync.dma_start(
                            out=out_ext[m * 128:(m + 1) * 128,
                                        n * QI:(n + 1) * QI],
                            in_=ob[:])
    nc.finalize()
    return nc


def make_in_maps(states, Wq, Wk, Wv, Wo, n_cores=N_CORES):
    b_, s_, d_ = states.shape
    R = b_ * s_
    HPC = d_ // n_cores
    bf = ml_dtypes.bfloat16
    statesT = np.ascontiguousarray(
        np.asarray(states, dtype=np.float32).reshape(R, d_).T).astype(bf)
    Wq = np.asarray(Wq, dtype=np.float32).astype(bf)
    Wk = np.asarray(Wk, dtype=np.float32).astype(bf)
    Wv = np.asarray(Wv, dtype=np.float32).astype(bf)
    Wo = np.ascontiguousarray(np.asarray(Wo, dtype=np.float32)).astype(bf)
    bias = build_bias()
    in_maps = []
    for c in range(n_cores):
        in_maps.append({
            "statesT": statesT,
            "wq": np.ascontiguousarray(Wq[:, c * HPC:(c + 1) * HPC]),
            "wk": np.ascontiguousarray(Wk[:, c * HPC:(c + 1) * HPC]),
            "wv": np.ascontiguousarray(Wv[:, c * HPC:(c + 1) * HPC]),
            "wo": Wo,
            "bias": bias,
        })
    return in_maps


_NC_CACHE = {}


def kernel(states, mask, Wq, Wk, Wv, Wo):
    """Full inputs -> full output [B, S, D]. mask is causal by construction
    (reference builds tril); causality is hardcoded on-chip."""
    from concourse.bass_utils import run_bass_kernel_spmd

    states = np.asarray(states, dtype=np.float32)
    b_, s_, d_ = states.shape
    key = (b_, s_, d_)
    if key not in _NC_CACHE:
        _NC_CACHE[key] = build(b_, s_, d_)
    nc = _NC_CACHE[key]
    in_maps = make_in_maps(states, Wq, Wk, Wv, Wo)
    res = run_bass_kernel_spmd(nc, in_maps, core_ids=list(range(N_CORES)))
    outs = [res.results[c]["out"] for c in range(N_CORES)]
    return np.concatenate(outs, axis=0).reshape(b_, s_, d_).astype(np.float32)


# revision 22
# speedup vs baseline: 1.2146x; 1.2146x over previous
"""Distributed attention layer kernel for 8 TRN2 NeuronCores.

Math (per reference): out = softmax_causal((x@Wq)(x@Wk)^T / 8) @ (x@Wv) @ Wo
with B=4, S=2048, D=1024, H=16 heads of dim 64.

Sharding: head tensor-parallel. Core c owns head pair (2c, 2c+1):
  - Wq/Wk/Wv column-sharded [1024, 128]; statesT replicated [1024, 8192].
  - Each core computes qT/kT/vT for its 2 heads, causal attention in
    S^T layout (kj on partitions, qi on free), softmax denominator via a
    ones-column appended to V (PV matmul row 64 = sum of probs).
  - ctx^T lands in AllToAll buffers (one per batch-half, overlapping the
    second half's compute) laid out so chunk j = [128 head cols, rows of
    output-core j]; after the A2A each core holds lhsT-ready blocks of
    ALL 1024 ctx columns for its rows.
  - Output projection: out_rows = sum_c ctxT_block_c.T @ Wo[128c:...]
    accumulated in PSUM; no AllReduce needed.

Matmul operands are bf16 (PE full rate); accumulation is fp32 in PSUM.
Scores for adjacent kj blocks share one multi-bank PSUM region so exp
runs as wide ScalarE ops (amortizing the ~352-cycle ACT overhead), and
diagonal blocks only compute their live column range.
"""

import ml_dtypes
import numpy as np

import concourse.bass as bass
import concourse.mybir as mybir
import concourse.tile as tile
from concourse import bacc
from concourse.masks import make_identity

F32 = mybir.dt.float32
BF16 = mybir.dt.bfloat16
NEG_BIAS = -1e35

B, S, D, H = 4, 2048, 1024, 16
HD = 64
N_CORES = 8


def build_bias(KJ=128):
    """Triangular bias for the 128-col partial strip of a diagonal block:
    bias[p, f] = 0 if p <= f else NEG_BIAS."""
    p = np.arange(KJ)[:, None]
    f = np.arange(KJ)[None, :]
    return np.where(p <= f, 0.0, NEG_BIAS).astype(np.float32)


def build(b_=B, s_=S, d_=D, n_cores=N_CORES):
    HPC = d_ // n_cores          # head cols per core (2 heads x 64)
    NH = HPC // HD               # heads per core
    R = b_ * s_                  # global rows
    Rc = R // n_cores            # output rows per core
    QI, KJ = 512, 128
    DT = d_ // 128               # D tiles (contraction)
    NB = QI // KJ                # 4
    SKJ = s_ // KJ               # kj blocks per (b, h)
    SQI = s_ // QI               # qi tiles per (b, h)
    n_half = 2 if (Rc // 2) % QI == 0 and b_ % 2 == 0 else 1
    HR = Rc // n_half            # rows per core per half
    RH = R // n_half             # global rows per half
    assert s_ % QI == 0 and Rc % QI == 0 and d_ % 128 == 0

    nc = bacc.Bacc(None, target_bir_lowering=False, debug=False)
    statesT = nc.declare_dram_parameter("statesT", [d_, R], BF16, isOutput=False)
    wq = nc.declare_dram_parameter("wq", [d_, HPC], BF16, isOutput=False)
    wk = nc.declare_dram_parameter("wk", [d_, HPC], BF16, isOutput=False)
    wv = nc.declare_dram_parameter("wv", [d_, HPC], BF16, isOutput=False)
    wo = nc.declare_dram_parameter("wo", [d_, d_], BF16, isOutput=False)
    bias_in = nc.declare_dram_parameter("bias", [KJ, KJ], F32, isOutput=False)
    out_ext = nc.declare_dram_parameter("out", [Rc, d_], F32, isOutput=True)

    SC = float(1.0 / np.sqrt(HD))
    EXP = mybir.ActivationFunctionType.Exp

    with tile.TileContext(nc) as tc:
        with tc.tile_pool(name="persist", bufs=1) as pp, \
             tc.tile_pool(name="dram", bufs=1, space="DRAM") as dram:
            a2a_in = [dram.tile([n_cores * HPC, HR], BF16, tag=f"a2a_in{hf}",
                                name=f"a2a_in{hf}")
                      for hf in range(n_half)]
            a2a_out = [dram.tile([n_cores * HPC, HR], BF16, tag=f"a2a_out{hf}",
                                 name=f"a2a_out{hf}")
                       for hf in range(n_half)]
            recip_d = [dram.tile([(b_ // n_half) * (d_ // n_cores // HD)
                                  * (s_ // 512), 512], BF16,
                                 tag=f"recip_d{hf}", name=f"recip_d{hf}")
                       for hf in range(n_half)]
            den_d = [dram.tile([1, (b_ // n_half) * (d_ // n_cores // HD)
                                * (s_ // 512) * 512], F32,
                               tag=f"den_d{hf}", name=f"den_d{hf}")
                     for hf in range(n_half)]

            qT = pp.tile([HPC, R], BF16, tag="qT")
            kT = pp.tile([HPC, R], BF16, tag="kT")
            vp = pp.tile([KJ, b_ * NH * SKJ, HD + 1], BF16, tag="vp")
            w_sb = pp.tile([128, 3, DT, HPC], BF16, tag="w_sb")
            wo_sb = pp.tile([128, DT, d_], BF16, tag="wo_sb")
            bias_sb = pp.tile([KJ, KJ], F32, tag="bias_sb")
            ident = pp.tile([128, 128], BF16, tag="ident")

            make_identity(nc, ident[:])
            nc.sync.dma_start(out=bias_sb[:], in_=bias_in[:, :])
            for i, w in enumerate([wq, wk, wv]):
                nc.sync.dma_start(
                    out=w_sb[:, i], in_=w[:, :].rearrange("(t p) c -> p t c", p=128))
            nc.sync.dma_start(
                out=wo_sb[:], in_=wo[:, :].rearrange("(t p) n -> p t n", p=128))

            # ---- Phase 1: QKV projections -> qT, kT, vT (all [HPC, R]) ----
            with tc.tile_pool(name="vT_pool", bufs=1) as vtp:
                vT = vtp.tile([HPC, R], BF16, tag="vT")
                with tc.tile_pool(name="st_in", bufs=3) as stp, \
                     tc.tile_pool(name="qkv_ps", bufs=3, space="PSUM") as qps:
                    dests = [qT, kT, vT]
                    for ci in range(R // QI):
                        st = stp.tile([128, DT, QI], BF16, tag="st")
                        nc.sync.dma_start(
                            out=st[:],
                            in_=statesT[:, ci * QI:(ci + 1) * QI]
                            .rearrange("(t p) f -> p t f", p=128))
                        for pi in range(3):
                            ps = qps.tile([128, QI], F32, tag="ps")
                            for dd in range(DT):
                                nc.tensor.matmul(
                                    ps[:], w_sb[:, pi, dd], st[:, dd],
                                    start=(dd == 0), stop=(dd == DT - 1))
                            nc.vector.tensor_copy(
                                dests[pi][:, ci * QI:(ci + 1) * QI], ps[:])

                # ---- Phase 2: V' blocks [KJ, HD+1] via PE transpose ----
                with tc.tile_pool(name="tp_ps", bufs=4, space="PSUM") as tpp:
                    nc.vector.memset(vp[:, :, HD], 1.0)
                    for bb in range(b_):
                        for h in range(NH):
                            for kj in range(SKJ):
                                blk = (bb * NH + h) * SKJ + kj
                                ps = tpp.tile([KJ, HD], BF16, tag="tp")
                                nc.tensor.transpose(
                                    ps[:],
                                    vT[h * HD:(h + 1) * HD,
                                       bb * s_ + kj * KJ: bb * s_ + (kj + 1) * KJ],
                                    ident[h * HD:(h + 1) * HD,
                                          h * HD:(h + 1) * HD])
                                nc.vector.tensor_copy(vp[:, blk, 0:HD], ps[:])

            # ---- Phase 3: causal attention per (b, h, qi) ----
            # Normalization (1/denominator) is deferred and batched per
            # batch-half: the per-qi DVE reciprocal on a single-partition
            # [1, 512] AP costs ~3.3us and serializes the pipeline; one
            # [16, 512] reciprocal uses 16 lanes for the same cost.
            HNT = (b_ // n_half) * NH * SQI  # ctx tiles per half
            with tc.tile_pool(name="sp_ps", bufs=3, space="PSUM") as spp, \
                 tc.tile_pool(name="ctx_ps", bufs=2, space="PSUM") as cps, \
                 tc.tile_pool(name="pt_sb", bufs=6) as ptp, \
                 tc.tile_pool(name="ctxu_sb", bufs=HNT + 2) as cup, \
                 tc.tile_pool(name="den_sb", bufs=2) as denp, \
                 tc.tile_pool(name="norm_sb", bufs=3) as nrmp, \
                 tc.tile_pool(name="misc_sb", bufs=3) as msp:
                half_state = {"den": None, "pending": []}

                def attn_bh(bb, h):
                    base = bb * s_
                    for qi in range(SQI):
                        ctx = cps.tile([HD + 1, QI], F32, tag="ctx")
                        q0 = base + qi * QI

                        def s_mm(out_ap, kj, coff, n):
                            nc.tensor.matmul(
                                out_ap,
                                kT[h * HD:(h + 1) * HD,
                                   base + kj * KJ: base + (kj + 1) * KJ],
                                qT[h * HD:(h + 1) * HD, q0 + coff: q0 + QI],
                                start=True, stop=True)

                        def pv_mm(kj, rhs_ap, coff, start, stop):
                            blk = (bb * NH + h) * SKJ + kj
                            nc.tensor.matmul(
                                ctx[:, coff:QI], vp[:, blk], rhs_ap,
                                start=start, stop=stop)

                        # diagonal blocks first (covers ctx fully via di=0),
                        # packed two per PSUM region: [di0|di1], [di2|di3]
                        for g in range(2):
                            di0, di1 = 2 * g, 2 * g + 1
                            n0, n1 = QI - KJ * di0, QI - KJ * di1
                            reg = spp.tile([128, 2 * QI], F32, tag="sp")
                            s_mm(reg[:, 0:n0], NB * qi + di0, KJ * di0, n0)
                            s_mm(reg[:, n0:n0 + n1], NB * qi + di1, KJ * di1, n1)
                            nc.vector.tensor_add(
                                reg[:, 0:KJ], reg[:, 0:KJ], bias_sb[:])
                            nc.vector.tensor_add(
                                reg[:, n0:n0 + KJ], reg[:, n0:n0 + KJ], bias_sb[:])
                            pt = ptp.tile([128, 2 * QI], BF16, tag="pt")
                            nc.scalar.activation(
                                pt[:, 0:n0 + n1], reg[:, 0:n0 + n1], EXP, scale=SC)
                            pv_mm(NB * qi + di0, pt[:, 0:n0], KJ * di0,
                                  start=(g == 0), stop=False)
                            pv_mm(NB * qi + di1, pt[:, n0:n0 + n1], KJ * di1,
                                  start=False, stop=(g == 1 and qi == 0))
                        # full blocks, paired
                        for kjp in range(2 * qi):
                            kja, kjb = 2 * kjp, 2 * kjp + 1
                            reg = spp.tile([128, 2 * QI], F32, tag="sp")
                            s_mm(reg[:, 0:QI], kja, 0, QI)
                            s_mm(reg[:, QI:2 * QI], kjb, 0, QI)
                            pt = ptp.tile([128, 2 * QI], BF16, tag="pt")
                            nc.scalar.activation(pt[:], reg[:], EXP, scale=SC)
                            pv_mm(kja, pt[:, 0:QI], 0, start=False, stop=False)
                            pv_mm(kjb, pt[:, QI:2 * QI], 0,
                                  start=False, stop=(kjp == 2 * qi - 1))
                        # epilogue: stash unnormalized ctx^T + denominator row
                        if half_state["den"] is None:
                            half_state["den"] = denp.tile(
                                [1, HNT * QI], F32, tag="den", name="den")
                        i = ((bb % (b_ // n_half)) * NH + h) * SQI + qi
                        ctxu = cup.tile([HD, QI], BF16, tag="ctxu", name="ctxu")
                        nc.vector.tensor_copy(ctxu[:], ctx[0:HD, :])
                        nc.vector.tensor_copy(
                            half_state["den"][0:1, i * QI:(i + 1) * QI],
                            ctx[HD:HD + 1, :])
                        g0 = base + qi * QI
                        hf, r = g0 // RH, g0 % RH
                        j, r0 = r // HR, r % HR
                        half_state["pending"].append((ctxu, hf, j, r0, i, h))

                def flush_half(hf):
                    den = half_state["den"]
                    nc.sync.dma_start(out=den_d[hf][0:1, :], in_=den[:])
                    den2 = nrmp.tile([HNT, QI], F32, tag="den2", name="den2")
                    nc.sync.dma_start(
                        out=den2[:],
                        in_=den_d[hf][0:1, :].rearrange(
                            "a (p f) -> (a p) f", p=HNT))
                    recipA = nrmp.tile([HNT, QI], BF16, tag="recipA",
                                       name="recipA")
                    with nc.allow_low_precision(
                            reason="softmax denom reciprocal to bf16"):
                        nc.vector.reciprocal(recipA[:], den2[:])
                    nc.sync.dma_start(out=recip_d[hf][:, :], in_=recipA[:])
                    for ctxu, hfi, j, r0, i, hh in half_state["pending"]:
                        assert hfi == hf
                        rb = nrmp.tile([HD, QI], BF16, tag="rb", name="rb")
                        nc.sync.dma_start(
                            out=rb[:],
                            in_=recip_d[hf][i:i + 1, :].to_broadcast([HD, QI]))
                        ctxT = msp.tile([HD, QI], BF16, tag="ctxT", name="ctxT")
                        nc.vector.tensor_mul(ctxT[:], ctxu[:], rb[:])
                        nc.sync.dma_start(
                            out=a2a_in[hf][j * HPC + hh * HD: j * HPC
                                           + (hh + 1) * HD, r0:r0 + QI],
                            in_=ctxT[:])
                    half_state["den"] = None
                    half_state["pending"] = []
                    nc.gpsimd.collective_compute(
                        "AllToAll", mybir.AluOpType.bypass,
                        replica_groups=[list(range(n_cores))],
                        ins=[a2a_in[hf][:].opt()], outs=[a2a_out[hf][:].opt()])

                for bb in range(b_):
                    for h in range(NH):
                        attn_bh(bb, h)
                    if bb % (b_ // n_half) == b_ // n_half - 1:
                        flush_half(bb // (b_ // n_half))

            # ---- Phase 4: output projection per half ----
            with tc.tile_pool(name="slab", bufs=1) as slp, \
                 tc.tile_pool(name="o_ps", bufs=3, space="PSUM") as ops, \
                 tc.tile_pool(name="o_sb", bufs=3) as osp:
                for hf in range(n_half):
                    slab = slp.tile([HPC, n_cores, HR], BF16, tag=f"slab{hf}")
                    nc.sync.dma_start(
                        out=slab[:],
                        in_=a2a_out[hf][:].rearrange("(c p) f -> p c f", p=HPC))
                    for m in range(HR // 128):
                        for n in range(d_ // QI):
                            ps = ops.tile([128, QI], F32, tag="o")
                            for c in range(n_cores):
                                nc.tensor.matmul(
                                    ps[:],
                                    slab[:, c, m * 128:(m + 1) * 128],
                                    wo_sb[:, c, n * QI:(n + 1) * QI],
                                    start=(c == 0), stop=(c == n_cores - 1))
                            ob = osp.tile([128, QI], F32, tag="ob")
                            nc.vector.tensor_copy(ob[:], ps[:])
                            nc.sync.dma_start(
                                out=out_ext[hf * HR + m * 128: hf * HR + (m + 1) * 128,
                                            n * QI:(n + 1) * QI],
                                in_=ob[:])
    nc.finalize()
    return nc


def make_in_maps(states, Wq, Wk, Wv, Wo, n_cores=N_CORES):
    b_, s_, d_ = states.shape
    R = b_ * s_
    HPC = d_ // n_cores
    bf = ml_dtypes.bfloat16
    statesT = np.ascontiguousarray(
        np.asarray(states, dtype=np.float32).reshape(R, d_).T).astype(bf)
    Wq = np.asarray(Wq, dtype=np.float32).astype(bf)
    Wk = np.asarray(Wk, dtype=np.float32).astype(bf)
    Wv = np.asarray(Wv, dtype=np.float32).astype(bf)
    Wo = np.ascontiguousarray(np.asarray(Wo, dtype=np.float32)).astype(bf)
    bias = build_bias()
    in_maps = []
    for c in range(n_cores):
        in_maps.append({
            "statesT": statesT,
            "wq": np.ascontiguousarray(Wq[:, c * HPC:(c + 1) * HPC]),
            "wk": np.ascontiguousarray(Wk[:, c * HPC:(c + 1) * HPC]),
            "wv": np.ascontiguousarray(Wv[:, c * HPC:(c + 1) * HPC]),
            "wo": Wo,
            "bias": bias,
        })
    return in_maps


def unshard(outs, b_, s_, d_, n_cores=N_CORES):
    """Core j's output rows are [half0: rows HR*j ...][half1: ...]."""
    R = b_ * s_
    Rc = R // n_cores
    QI = 512
    n_half = 2 if (Rc // 2) % QI == 0 and b_ % 2 == 0 else 1
    HR = Rc // n_half
    RH = R // n_half
    full = np.empty((R, d_), dtype=np.float32)
    for j in range(n_cores):
        for hf in range(n_half):
            full[hf * RH + j * HR: hf * RH + (j + 1) * HR] = \
                outs[j][hf * HR:(hf + 1) * HR]
    return full.reshape(b_, s_, d_)


_NC_CACHE = {}


def kernel(states, mask, Wq, Wk, Wv, Wo):
    """Full inputs -> full output [B, S, D]. mask is causal by construction
    (reference builds tril); causality is hardcoded on-chip."""
    from concourse.bass_utils import run_bass_kernel_spmd

    states = np.asarray(states, dtype=np.float32)
    b_, s_, d_ = states.shape
    key = (b_, s_, d_)
    if key not in _NC_CACHE:
        _NC_CACHE[key] = build(b_, s_, d_)
    nc = _NC_CACHE[key]
    in_maps = make_in_maps(states, Wq, Wk, Wv, Wo)
    res = run_bass_kernel_spmd(nc, in_maps, core_ids=list(range(N_CORES)))
    outs = [res.results[c]["out"] for c in range(N_CORES)]
    return unshard(outs, b_, s_, d_).astype(np.float32)
